# revision 32
# baseline (speedup 1.0000x reference)
"""Trainium2 Bass kernel for the DataReloadingQNN problem.

Math: layers 0..4 plus the shared RZ/RY/RZ of layer 5 collapse into one
fixed state w (params only).  The data gates are RY(x_q) = c_q I + s_q J_q
with J a signed permutation, all commuting.  Qubits 0..7 are contracted
by a dense matmul T = W_lo @ U with
    W_lo[b, m] = tensor product of 8 [cos,sin] pairs  (m in [0,256)),
    U[m, :]   = (P J^{(m)} w) re/im-interleaved, P = CNOT chain,
and the peeled qubits 8,9,10 are applied as per-sample rotations
    T <- c_q T + s_q * sign_q ( T[col ^ M_q] ).
Columns are relabeled host-side by a linear GF(2) map chosen so each
peeled rotation is a single y-bit flip with sign = that bit (bits 4,3,2
-> col blocks 64/32/16, contiguous halves).

Device schedule per sample-tile (128 samples):
  - for NPE of the 8 tiles, the q=8 rotation is folded into the matmul
    (signed-permuted copy U8 plus weight variants c8*W, s8*W -> K=512,
    32 matmuls); for the rest it runs on VectorE (K=256, 16 matmuls)
  - ScalarE drains PSUM as two scaled copies (c9*P, s9*P), which is the
    q=9 rotation's prep; VectorE finishes it with two tensor_tensor
    half-adds, then does q=10 (and q=8 on the non-folded tiles) the same
    way (tensor_scalar 4x + tensor_tensor 2x)
  - output is written bf16 in y-order; the host upcasts and unpermutes
W_lo (a 0.2%-of-FLOPs prefix) plus the per-sample cos/sin coefficients
are prepared on the host, pre-scaled and pre-transposed, so the device
does no W build, no transposes and needs no trig tables.

Per core: 1024 samples = 8 tiles of 128.  Inputs sharded batch-wise
across 8 cores; U replicated.
"""
import numpy as np
import ml_dtypes

import concourse.bass as bass
import concourse.bacc as bacc
import concourse.tile as tile
from concourse import mybir
from concourse.bass_utils import run_bass_kernel_spmd

N = 11
DIM = 2048
BATCH = 8192
NCORES = 8
BSH = BATCH // NCORES          # 1024 samples per core
NTILES = BSH // 128            # 8 sample-tiles per core
KLO = 8                        # qubits contracted in the matmul
NU = 1 << KLO                  # 256 rows of U
W2 = 2 * DIM                   # 4096 output columns (re/im interleaved)
NPE = 8                        # tiles with q=8 folded into the matmul
F32 = mybir.dt.float32
BF16 = mybir.dt.bfloat16

ADD = mybir.AluOpType.add
SUB = mybir.AluOpType.subtract

# q=8 on PE for the last NPE tiles: the early tiles' VectorE q=8 load
# (and their slower PSUM drain) overlaps the U8 DMA, and the trailing
# PE-folded tiles stream matmuls without ACT-paced stalls
TILE_PE = [t >= NTILES - NPE for t in range(NTILES)]

# ---------------------------------------------------------------- host math


def _rz(phi):
    e = np.exp(-0.5j * phi)
    return np.array([[e, 0], [0, np.conj(e)]], dtype=np.complex128)


def _ry(theta):
    t = 0.5 * theta
    c, s = np.cos(t), np.sin(t)
    return np.array([[c, -s], [s, c]], dtype=np.complex128)


def _apply_1q_rows(rows, U, q):
    R = rows.shape[0]
    st = rows.reshape(R, 2 ** q, 2, 2 ** (N - 1 - q))
    st = np.einsum('ab,rxby->rxay', U, st)
    return st.reshape(R, DIM)


def _apply_cnot_rows(rows, c):
    R = rows.shape[0]
    st = rows.reshape(R, 2 ** c, 2, 2, 2 ** (N - 2 - c))
    st = np.stack([st[:, :, 0], st[:, :, 1, ::-1]], axis=2)
    return st.reshape(R, DIM)


def _y_of_x():
    """Column relabeling y = R x: y0=x3, y1=x4, y2=x0^x1, y3=x1^x2,
    y4=x2^x3, y5..10 = x5..x10 (bit i of the state index = 2^i)."""
    x = np.arange(DIM)
    x0, x1 = x & 1, (x >> 1) & 1
    x2, x3 = (x >> 2) & 1, (x >> 3) & 1
    x4 = (x >> 4) & 1
    return ((x & ~np.int64(31)) | (x3 << 0) | (x4 << 1)
            | ((x0 ^ x1) << 2) | ((x1 ^ x2) << 3) | ((x2 ^ x3) << 4))


def _x_of_y():
    y = _y_of_x()
    inv = np.empty(DIM, dtype=np.int64)
    inv[y] = np.arange(DIM)
    return inv


def build_u_matrices(params):
    """(6,11,3) f32 -> (Uy, U8), each (256, 4096) f64 in y-order.
    U8 is the signed bit-4-flip permutation of Uy (folds the q=8 gate)."""
    p = params.astype(np.float64)
    v = np.zeros((1, DIM), dtype=np.complex128)
    v[0, 0] = 1.0
    for l in range(5):
        for q in range(N):
            v = _apply_1q_rows(v, _rz(p[l, q, 0]), q)
            v = _apply_1q_rows(v, _ry(p[l, q, 1]), q)
            v = _apply_1q_rows(v, _rz(p[l, q, 2]), q)
        for c in range(N - 1):
            v = _apply_cnot_rows(v, c)
    for q in range(N):
        B = _rz(p[5, q, 2]) @ _ry(p[5, q, 1]) @ _rz(p[5, q, 0])
        v = _apply_1q_rows(v, B, q)

    # rows over J-subsets of qubits 0..7 (bit b of m <-> qubit b)
    rows = v
    idx = np.arange(DIM)
    for q in range(KLO):
        m = 1 << (N - 1 - q)
        sgn = np.where(idx & m, 1.0, -1.0)
        rows = np.concatenate([rows, sgn * rows[:, idx ^ m]], axis=0)

    # fold CNOT-chain permutation, then relabel columns to y-order
    g = np.arange(DIM)[None, :]
    for c in range(N - 1):
        g = _apply_cnot_rows(g.astype(np.float64), c).astype(np.int64)
    rows = rows[:, g[0]][:, _x_of_y()]

    # fold the q=8 rotation: U8 = sign(y bit 4) * Uy[:, y ^ 16]
    yy = np.arange(DIM)
    sgn8 = np.where((yy >> 4) & 1, 1.0, -1.0)
    rows8 = sgn8[None, :] * rows[:, yy ^ 16]

    def interleave(r):
        U = np.empty((NU, W2), dtype=np.float64)
        U[:, 0::2] = r.real
        U[:, 1::2] = r.imag
        return U

    return interleave(rows), interleave(rows8)


def build_weights(X):
    """Per-sample host prep: cos/sin of x/2, the W_lo tensor product and
    the pre-transposed weight variants [W^T, (c8 W)^T, (s8 W)^T], each
    split into two K-chunks of 128, plus the rotation coefficients."""
    c = np.cos(0.5 * X).astype(np.float64)   # (B, 11)
    s = np.sin(0.5 * X).astype(np.float64)
    B = X.shape[0]
    W = np.ones((B, 1), dtype=np.float64)
    for q in range(KLO):
        W = np.concatenate([W * c[:, q:q + 1], W * s[:, q:q + 1]], axis=1)

    wt = np.empty((3, 2, 128, B), dtype=ml_dtypes.bfloat16)
    for vi, scale in enumerate((np.ones(B), c[:, 8], s[:, 8])):
        Wv = (W * scale[:, None]).astype(ml_dtypes.bfloat16)
        wt[vi, 0] = Wv[:, :128].T
        wt[vi, 1] = Wv[:, 128:].T

    ntile = B // 128
    coef = np.empty((128, ntile * 6), dtype=np.float32)
    for t in range(ntile):
        blk = slice(t * 128, (t + 1) * 128)
        for j, arr in enumerate((c[:, 8], s[:, 8], c[:, 9], s[:, 9],
                                 c[:, 10], s[:, 10])):
            coef[:, t * 6 + j] = arr[blk]
    return wt, coef


# ------------------------------------------------------------- bass kernel


def _rot_tt(nc, dst, u, w, block):
    """dst_hi = u_hi + w_lo ; dst_lo = u_lo - w_hi  per block (APs)."""
    H = block // 2
    vd = dst.rearrange("p (g u) -> p g u", u=block)
    vu = u.rearrange("p (g u) -> p g u", u=block)
    vw = w.rearrange("p (g u) -> p g u", u=block)
    nc.vector.tensor_tensor(vd[:, :, H:], vu[:, :, H:], vw[:, :, :H], ADD)
    nc.vector.tensor_tensor(vd[:, :, :H], vu[:, :, :H], vw[:, :, H:], SUB)


def build_kernel():
    nc = bacc.Bacc()
    wt_d = nc.dram_tensor("wt", (3, 2, 128, BSH), BF16, kind="ExternalInput")
    cf_d = nc.dram_tensor("cf", (128, NTILES * 6), F32, kind="ExternalInput")
    u_d = nc.dram_tensor("u", (4, 128, W2), BF16, kind="ExternalInput")
    out_d = nc.dram_tensor("out", (BSH, W2), BF16, kind="ExternalOutput")

    with tile.TileContext(nc) as tc:
        with (
            tc.tile_pool(name="const", bufs=1) as const_pool,
            tc.tile_pool(name="rot", bufs=2) as rot_pool,
            tc.tile_pool(name="pmm", bufs=2, space=bass.MemorySpace.PSUM) as pmm_pool,
        ):
            # DMA schedule: tile0 (a PE tile) needs cf, wt1x/wt2x, Uy and
            # U8.  Split the big transfers across all three DMA queues.
            cf_sb = const_pool.tile([128, NTILES * 6], F32)
            nc.sync.dma_start(cf_sb[:], cf_d[:])
            wt_of = {}
            for vi in range(3):
                for k in range(2):
                    w = const_pool.tile([128, BSH], BF16, tag=f"wt{vi}{k}",
                                        name=f"wt{vi}{k}")
                    wt_of[(vi, k)] = w
            # U tiles per (matrix k, half-row): matmuls of half-row hw only
            # need that half's 2048 cols.  Transfer order favours what the
            # first matmul group reads (h0 halves, then wt, then h1).
            u_sb = {}
            for k in range(4):
                for hw in range(2):
                    ut = const_pool.tile([128, 2048], BF16, tag=f"u{k}h{hw}",
                                         name=f"u{k}h{hw}")
                    u_sb[(k, hw)] = ut
            nc.sync.dma_start(wt_of[(1, 0)][:], wt_d[1, 0])
            nc.scalar.dma_start(wt_of[(1, 1)][:], wt_d[1, 1])
            nc.sync.dma_start(u_sb[(0, 0)][:], u_d[0, :, 0:2048])
            nc.scalar.dma_start(u_sb[(1, 0)][:], u_d[1, :, 0:2048])
            nc.sync.dma_start(wt_of[(2, 0)][:], wt_d[2, 0])
            nc.scalar.dma_start(wt_of[(2, 1)][:], wt_d[2, 1])
            nc.sync.dma_start(u_sb[(0, 1)][:], u_d[0, :, 2048:])
            nc.scalar.dma_start(u_sb[(1, 1)][:], u_d[1, :, 2048:])
            if NPE < NTILES:
                nc.sync.dma_start(wt_of[(0, 0)][:], wt_d[0, 0])
                nc.scalar.dma_start(wt_of[(0, 1)][:], wt_d[0, 1])
            # U8 = sign(col bit 5) * Uy[col ^ 32]: derive on device
            # instead of spending 2 MB of HBM reads on it
            for k in range(2):
                for hw in range(2):
                    vs = u_sb[(k, hw)][:].rearrange("p (g u) -> p g u", u=64)
                    vd = u_sb[(k + 2, hw)][:].rearrange("p (g u) -> p g u",
                                                        u=64)
                    nc.vector.tensor_copy(vd[:, :, 32:], vs[:, :, :32])
                    nc.vector.tensor_scalar_mul(vd[:, :, :32],
                                                vs[:, :, 32:], -1.0)

            def cf(t, j):
                return cf_sb[:, t * 6 + j:t * 6 + j + 1]

            for t in range(NTILES):
                ts = slice(t * 128, (t + 1) * 128)
                pe8 = TILE_PE[t]
                if pe8:
                    variants = ((1, 0, 0), (1, 1, 1), (2, 0, 2), (2, 1, 3))
                else:
                    variants = ((0, 0, 0), (0, 1, 1))
                nv = len(variants)

                u9b = rot_pool.tile([128, W2], BF16, tag="u9b")
                w9 = rot_pool.tile([128, W2], BF16, tag="w9")
                T2 = rot_pool.tile([128, W2], BF16, tag="T2")
                ua = rot_pool.tile([128, W2], BF16, tag="ua")
                wa = rot_pool.tile([128, W2], BF16, tag="wa")
                T3 = rot_pool.tile([128, W2], BF16, tag="T3")
                if not pe8:
                    T4 = rot_pool.tile([128, W2], BF16, tag="T4")
                    ub = rot_pool.tile([128, W2], BF16, tag="ub")
                    wb = rot_pool.tile([128, W2], BF16, tag="wb")

                nseg = 2 if t < NTILES - 2 else 4
                for hw in range(2):          # half-row pipeline
                    pmm = pmm_pool.tile([128, 2048], F32, tag="pmm",
                                        name="pmm")
                    for vi, (wvar, k, ui) in enumerate(variants):
                        wop = wt_of[(wvar, k)]
                        for h in range(4):
                            nc.tensor.matmul(
                                pmm[:, h * 512:(h + 1) * 512],
                                wop[:, ts],
                                u_sb[(ui, hw)][:, h * 512:(h + 1) * 512],
                                start=(vi == 0), stop=(vi == nv - 1))
                    hs = slice(hw * 2048, (hw + 1) * 2048)
                    # q=9 prep fused into the PSUM drain (finer on the
                    # last tiles to shorten the tail).  VectorE drains the
                    # final 256 cols so the PSUM buffer frees before the
                    # ScalarE pass ends and the next matmul group never
                    # waits.
                    if nseg == 2:
                        pv = pmm[:, 0:1792]
                        ds_ = slice(hw * 2048, hw * 2048 + 1792)
                        nc.scalar.mul(u9b[:, ds_], pv, cf(t, 2))
                        nc.scalar.mul(w9[:, ds_], pv, cf(t, 3))
                        pv2 = pmm[:, 1792:2048]
                        ds2 = slice(hw * 2048 + 1792, (hw + 1) * 2048)
                        nc.vector.tensor_scalar_mul(u9b[:, ds2], pv2,
                                                    cf(t, 2))
                        nc.vector.tensor_scalar_mul(w9[:, ds2], pv2,
                                                    cf(t, 3))
                    else:
                        for dg in range(nseg // 2):
                            dw = 2048 // (nseg // 2)
                            ds_ = slice(hw * 2048 + dg * dw,
                                        hw * 2048 + (dg + 1) * dw)
                            pv = pmm[:, dg * dw:(dg + 1) * dw]
                            nc.scalar.mul(u9b[:, ds_], pv, cf(t, 2))
                            nc.scalar.mul(w9[:, ds_], pv, cf(t, 3))

                    # finer segments at the very end shorten the tail
                    segw = 2048 // (nseg // 2)
                    for sg in range(nseg // 2):
                        ss = slice(hw * 2048 + sg * segw,
                                   hw * 2048 + (sg + 1) * segw)
                        # q=9 (block 32)
                        _rot_tt(nc, T2[:, ss], u9b[:, ss], w9[:, ss], 32)
                        if not pe8:
                            # q=8 on VectorE (block 64)
                            nc.vector.tensor_scalar_mul(ub[:, ss], T2[:, ss],
                                                        cf(t, 0))
                            nc.vector.tensor_scalar_mul(wb[:, ss], T2[:, ss],
                                                        cf(t, 1))
                            _rot_tt(nc, T4[:, ss], ub[:, ss], wb[:, ss], 64)
                            src = T4
                        else:
                            src = T2
                        # q=10 (block 16)
                        nc.vector.tensor_scalar_mul(ua[:, ss], src[:, ss],
                                                    cf(t, 4))
                        nc.vector.tensor_scalar_mul(wa[:, ss], src[:, ss],
                                                    cf(t, 5))
                        _rot_tt(nc, T3[:, ss], ua[:, ss], wa[:, ss], 16)
                        oq = nc.gpsimd if (t + hw + sg) % 2 else nc.sync
                        oq.dma_start(out_d[ts, ss], T3[:, ss])
    nc.finalize()
    return nc


# ----------------------------------------------------------------- driver

_CACHE = {}


def kernel(X, params):
    X = np.ascontiguousarray(np.asarray(X, dtype=np.float32))
    params = np.asarray(params, dtype=np.float32)

    Uy, U8 = build_u_matrices(params)
    u_bf = np.ascontiguousarray(np.stack([
        Uy[:128], Uy[128:], U8[:128], U8[128:],
    ]).astype(ml_dtypes.bfloat16))
    wt, coef = build_weights(X)

    if "nc" not in _CACHE:
        _CACHE["nc"] = build_kernel()
    nc = _CACHE["nc"]

    ncols = BATCH // 128
    coef3 = coef.reshape(128, ncols, 6)
    in_maps = []
    for c in range(NCORES):
        bs = slice(c * BSH, (c + 1) * BSH)
        in_maps.append({
            "wt": np.ascontiguousarray(wt[:, :, :, bs]),
            "cf": np.ascontiguousarray(
                coef3[:, c * NTILES:(c + 1) * NTILES].reshape(
                    128, NTILES * 6)),
            "u": u_bf,
        })
    res = run_bass_kernel_spmd(nc, in_maps, list(range(NCORES)))
    out = np.concatenate([res.results[c]["out"] for c in range(NCORES)],
                         axis=0)
    # device columns are y-ordered; out[x] = dev[y(x)]
    out = out.astype(np.float32).reshape(BATCH, DIM, 2)
    return np.ascontiguousarray(out[:, _y_of_x(), :])


# revision 33
# speedup vs baseline: 1.1035x; 1.1035x over previous
"""Trainium2 Bass kernel for the DataReloadingQNN problem.

Math: layers 0..4 plus the shared RZ/RY/RZ of layer 5 collapse into one
fixed state w (params only).  The data gates are RY(x_q) = c_q I + s_q J_q
with J a signed permutation, all commuting.  Qubits 0..7 are contracted
by a dense matmul T = W_lo @ U with
    W_lo[b, m] = tensor product of 8 [cos,sin] pairs  (m in [0,256)),
    U[m, :]   = (P J^{(m)} w) re/im-interleaved, P = CNOT chain,
and the peeled qubits 8,9,10 are applied as per-sample rotations
    T <- c_q T + s_q * sign_q ( T[col ^ M_q] ).
Columns are relabeled host-side by a linear GF(2) map chosen so each
peeled rotation is a single y-bit flip with sign = that bit (bits 4,3,2
-> col blocks 64/32/16, contiguous halves).

Device schedule per sample-tile (128 samples):
  - for NPE of the 8 tiles, the q=8 rotation is folded into the matmul
    (signed-permuted copy U8 plus weight variants c8*W, s8*W -> K=512,
    32 matmuls); for the rest it runs on VectorE (K=256, 16 matmuls)
  - ScalarE drains PSUM as two scaled copies (c9*P, s9*P), which is the
    q=9 rotation's prep; VectorE finishes it with two tensor_tensor
    half-adds, then does q=10 (and q=8 on the non-folded tiles) the same
    way (tensor_scalar 4x + tensor_tensor 2x)
  - output is written bf16 in y-order; the host upcasts and unpermutes
W_lo (a 0.2%-of-FLOPs prefix) plus the per-sample cos/sin coefficients
are prepared on the host, pre-scaled and pre-transposed, so the device
does no W build, no transposes and needs no trig tables.

Per core: 1024 samples = 8 tiles of 128.  Inputs sharded batch-wise
across 8 cores; U replicated.
"""
import numpy as np
import ml_dtypes

import concourse.bass as bass
import concourse.bacc as bacc
import concourse.tile as tile
from concourse import mybir
from concourse.bass_utils import run_bass_kernel_spmd

N = 11
DIM = 2048
BATCH = 8192
NCORES = 8
BSH = BATCH // NCORES          # 1024 samples per core
NTILES = BSH // 128            # 8 sample-tiles per core
KLO = 8                        # qubits contracted in the matmul
NU = 1 << KLO                  # 256 rows of U
W2 = 2 * DIM                   # 4096 output columns (re/im interleaved)
NPE = 8                        # tiles with q=8 folded into the matmul
F32 = mybir.dt.float32
BF16 = mybir.dt.bfloat16

ADD = mybir.AluOpType.add
SUB = mybir.AluOpType.subtract

# q=8 on PE for the last NPE tiles: the early tiles' VectorE q=8 load
# (and their slower PSUM drain) overlaps the U8 DMA, and the trailing
# PE-folded tiles stream matmuls without ACT-paced stalls
TILE_PE = [t >= NTILES - NPE for t in range(NTILES)]

# ---------------------------------------------------------------- host math


def _rz(phi):
    e = np.exp(-0.5j * phi)
    return np.array([[e, 0], [0, np.conj(e)]], dtype=np.complex128)


def _ry(theta):
    t = 0.5 * theta
    c, s = np.cos(t), np.sin(t)
    return np.array([[c, -s], [s, c]], dtype=np.complex128)


def _apply_1q_rows(rows, U, q):
    R = rows.shape[0]
    st = rows.reshape(R, 2 ** q, 2, 2 ** (N - 1 - q))
    st = np.einsum('ab,rxby->rxay', U, st)
    return st.reshape(R, DIM)


def _apply_cnot_rows(rows, c):
    R = rows.shape[0]
    st = rows.reshape(R, 2 ** c, 2, 2, 2 ** (N - 2 - c))
    st = np.stack([st[:, :, 0], st[:, :, 1, ::-1]], axis=2)
    return st.reshape(R, DIM)


def _y_of_x():
    """Column relabeling y = R x: y0=x3, y1=x4, y2=x0^x1, y3=x1^x2,
    y4=x2^x3, y5..10 = x5..x10 (bit i of the state index = 2^i)."""
    x = np.arange(DIM)
    x0, x1 = x & 1, (x >> 1) & 1
    x2, x3 = (x >> 2) & 1, (x >> 3) & 1
    x4 = (x >> 4) & 1
    return ((x & ~np.int64(31)) | (x3 << 0) | (x4 << 1)
            | ((x0 ^ x1) << 2) | ((x1 ^ x2) << 3) | ((x2 ^ x3) << 4))


def _x_of_y():
    y = _y_of_x()
    inv = np.empty(DIM, dtype=np.int64)
    inv[y] = np.arange(DIM)
    return inv


def build_u_matrices(params):
    """(6,11,3) f32 -> (Uy, U8), each (256, 4096) f64 in y-order.
    U8 is the signed bit-4-flip permutation of Uy (folds the q=8 gate)."""
    p = params.astype(np.float64)
    v = np.zeros((1, DIM), dtype=np.complex128)
    v[0, 0] = 1.0
    for l in range(5):
        for q in range(N):
            v = _apply_1q_rows(v, _rz(p[l, q, 0]), q)
            v = _apply_1q_rows(v, _ry(p[l, q, 1]), q)
            v = _apply_1q_rows(v, _rz(p[l, q, 2]), q)
        for c in range(N - 1):
            v = _apply_cnot_rows(v, c)
    for q in range(N):
        B = _rz(p[5, q, 2]) @ _ry(p[5, q, 1]) @ _rz(p[5, q, 0])
        v = _apply_1q_rows(v, B, q)

    # rows over J-subsets of qubits 0..7 (bit b of m <-> qubit b)
    rows = v
    idx = np.arange(DIM)
    for q in range(KLO):
        m = 1 << (N - 1 - q)
        sgn = np.where(idx & m, 1.0, -1.0)
        rows = np.concatenate([rows, sgn * rows[:, idx ^ m]], axis=0)

    # fold CNOT-chain permutation, then relabel columns to y-order
    g = np.arange(DIM)[None, :]
    for c in range(N - 1):
        g = _apply_cnot_rows(g.astype(np.float64), c).astype(np.int64)
    rows = rows[:, g[0]][:, _x_of_y()]

    # fold the q=8 rotation: U8 = sign(y bit 4) * Uy[:, y ^ 16]
    yy = np.arange(DIM)
    sgn8 = np.where((yy >> 4) & 1, 1.0, -1.0)
    rows8 = sgn8[None, :] * rows[:, yy ^ 16]

    def interleave(r):
        U = np.empty((NU, W2), dtype=np.float64)
        U[:, 0::2] = r.real
        U[:, 1::2] = r.imag
        return U

    return interleave(rows), interleave(rows8)


def build_weights(X):
    """Per-sample host prep: cos/sin of x/2, the W_lo tensor product and
    the pre-transposed weight variants [W^T, (c8 W)^T, (s8 W)^T], each
    split into two K-chunks of 128, plus the rotation coefficients."""
    c = np.cos(0.5 * X).astype(np.float64)   # (B, 11)
    s = np.sin(0.5 * X).astype(np.float64)
    B = X.shape[0]
    W = np.ones((B, 1), dtype=np.float64)
    for q in range(KLO):
        W = np.concatenate([W * c[:, q:q + 1], W * s[:, q:q + 1]], axis=1)

    wt = np.empty((3, 2, 128, B), dtype=ml_dtypes.bfloat16)
    for vi, scale in enumerate((np.ones(B), c[:, 8], s[:, 8])):
        Wv = (W * scale[:, None]).astype(ml_dtypes.bfloat16)
        wt[vi, 0] = Wv[:, :128].T
        wt[vi, 1] = Wv[:, 128:].T

    ntile = B // 128
    coef = np.empty((128, ntile * 6), dtype=np.float32)
    for t in range(ntile):
        blk = slice(t * 128, (t + 1) * 128)
        for j, arr in enumerate((c[:, 8], s[:, 8], c[:, 9], s[:, 9],
                                 c[:, 10], s[:, 10])):
            coef[:, t * 6 + j] = arr[blk]
    return wt, coef


# ------------------------------------------------------------- bass kernel


def _rot_tt(nc, dst, u, w, block):
    """dst_hi = u_hi + w_lo ; dst_lo = u_lo - w_hi  per block (APs)."""
    H = block // 2
    vd = dst.rearrange("p (g u) -> p g u", u=block)
    vu = u.rearrange("p (g u) -> p g u", u=block)
    vw = w.rearrange("p (g u) -> p g u", u=block)
    nc.vector.tensor_tensor(vd[:, :, H:], vu[:, :, H:], vw[:, :, :H], ADD)
    nc.vector.tensor_tensor(vd[:, :, :H], vu[:, :, :H], vw[:, :, H:], SUB)


def build_kernel():
    nc = bacc.Bacc()
    wt_d = nc.dram_tensor("wt", (3, 2, 128, BSH), BF16, kind="ExternalInput")
    cf_d = nc.dram_tensor("cf", (128, NTILES * 6), F32, kind="ExternalInput")
    u_d = nc.dram_tensor("u", (4, 128, W2), BF16, kind="ExternalInput")
    out_d = nc.dram_tensor("out", (BSH, W2), BF16, kind="ExternalOutput")

    with tile.TileContext(nc) as tc:
        with (
            tc.tile_pool(name="const", bufs=1) as const_pool,
            tc.tile_pool(name="rot", bufs=2) as rot_pool,
            tc.tile_pool(name="pmm", bufs=2, space=bass.MemorySpace.PSUM) as pmm_pool,
        ):
            # DMA schedule: tile0 (a PE tile) needs cf, wt1x/wt2x, Uy and
            # U8.  Split the big transfers across all three DMA queues.
            cf_sb = const_pool.tile([128, NTILES * 6], F32)
            nc.sync.dma_start(cf_sb[:], cf_d[:])
            wt_of = {}
            for vi in range(3):
                for k in range(2):
                    w = const_pool.tile([128, BSH], BF16, tag=f"wt{vi}{k}",
                                        name=f"wt{vi}{k}")
                    wt_of[(vi, k)] = w
            # U tiles per (matrix k, half-row): matmuls of half-row hw only
            # need that half's 2048 cols.  Transfer order favours what the
            # first matmul group reads (h0 halves, then wt, then h1).
            u_sb = {}
            for k in range(4):
                for hw in range(2):
                    ut = const_pool.tile([128, 2048], BF16, tag=f"u{k}h{hw}",
                                         name=f"u{k}h{hw}")
                    u_sb[(k, hw)] = ut
            nc.sync.dma_start(wt_of[(1, 0)][:], wt_d[1, 0])
            nc.scalar.dma_start(wt_of[(1, 1)][:], wt_d[1, 1])
            nc.sync.dma_start(u_sb[(0, 0)][:], u_d[0, :, 0:2048])
            nc.scalar.dma_start(u_sb[(1, 0)][:], u_d[1, :, 0:2048])
            nc.sync.dma_start(wt_of[(2, 0)][:], wt_d[2, 0])
            nc.scalar.dma_start(wt_of[(2, 1)][:], wt_d[2, 1])
            nc.sync.dma_start(u_sb[(0, 1)][:], u_d[0, :, 2048:])
            nc.scalar.dma_start(u_sb[(1, 1)][:], u_d[1, :, 2048:])
            if NPE < NTILES:
                nc.sync.dma_start(wt_of[(0, 0)][:], wt_d[0, 0])
                nc.scalar.dma_start(wt_of[(0, 1)][:], wt_d[0, 1])
            # U8 = sign(col bit 5) * Uy[col ^ 32]: derive on device
            # instead of spending 2 MB of HBM reads on it
            for k in range(2):
                for hw in range(2):
                    vs = u_sb[(k, hw)][:].rearrange("p (g u) -> p g u", u=64)
                    vd = u_sb[(k + 2, hw)][:].rearrange("p (g u) -> p g u",
                                                        u=64)
                    nc.vector.tensor_copy(vd[:, :, 32:], vs[:, :, :32])
                    nc.vector.tensor_scalar_mul(vd[:, :, :32],
                                                vs[:, :, 32:], -1.0)

            def cf(t, j):
                return cf_sb[:, t * 6 + j:t * 6 + j + 1]

            for t in range(NTILES):
                ts = slice(t * 128, (t + 1) * 128)
                pe8 = TILE_PE[t]
                if pe8:
                    variants = ((1, 0, 0), (1, 1, 1), (2, 0, 2), (2, 1, 3))
                else:
                    variants = ((0, 0, 0), (0, 1, 1))
                nv = len(variants)

                u9b = rot_pool.tile([128, W2], BF16, tag="u9b")
                w9 = rot_pool.tile([128, W2], BF16, tag="w9")
                T2 = rot_pool.tile([128, W2], BF16, tag="T2")
                ua = rot_pool.tile([128, W2], BF16, tag="ua")
                wa = rot_pool.tile([128, W2], BF16, tag="wa")
                T3 = rot_pool.tile([128, W2], BF16, tag="T3")
                if not pe8:
                    T4 = rot_pool.tile([128, W2], BF16, tag="T4")
                    ub = rot_pool.tile([128, W2], BF16, tag="ub")
                    wb = rot_pool.tile([128, W2], BF16, tag="wb")

                nseg = 2 if t < NTILES - 1 else 4
                for hw in range(2):          # half-row pipeline
                    pmm = pmm_pool.tile([128, 2048], F32, tag="pmm",
                                        name="pmm")
                    for vi, (wvar, k, ui) in enumerate(variants):
                        wop = wt_of[(wvar, k)]
                        for h in range(4):
                            nc.tensor.matmul(
                                pmm[:, h * 512:(h + 1) * 512],
                                wop[:, ts],
                                u_sb[(ui, hw)][:, h * 512:(h + 1) * 512],
                                start=(vi == 0), stop=(vi == nv - 1))
                    hs = slice(hw * 2048, (hw + 1) * 2048)
                    # q=9 prep fused into the PSUM drain (finer on the
                    # last tile to shorten the tail)
                    for dg in range(nseg // 2):
                        dw = 2048 // (nseg // 2)
                        ds_ = slice(hw * 2048 + dg * dw,
                                    hw * 2048 + (dg + 1) * dw)
                        pv = pmm[:, dg * dw:(dg + 1) * dw]
                        nc.scalar.mul(u9b[:, ds_], pv, cf(t, 2))
                        nc.scalar.mul(w9[:, ds_], pv, cf(t, 3))

                    # finer segments at the very end shorten the tail
                    segw = 2048 // (nseg // 2)
                    for sg in range(nseg // 2):
                        ss = slice(hw * 2048 + sg * segw,
                                   hw * 2048 + (sg + 1) * segw)
                        # q=9 (block 32)
                        _rot_tt(nc, T2[:, ss], u9b[:, ss], w9[:, ss], 32)
                        if not pe8:
                            # q=8 on VectorE (block 64)
                            nc.vector.tensor_scalar_mul(ub[:, ss], T2[:, ss],
                                                        cf(t, 0))
                            nc.vector.tensor_scalar_mul(wb[:, ss], T2[:, ss],
                                                        cf(t, 1))
                            _rot_tt(nc, T4[:, ss], ub[:, ss], wb[:, ss], 64)
                            src = T4
                        else:
                            src = T2
                        # q=10 (block 16)
                        nc.vector.tensor_scalar_mul(ua[:, ss], src[:, ss],
                                                    cf(t, 4))
                        nc.vector.tensor_scalar_mul(wa[:, ss], src[:, ss],
                                                    cf(t, 5))
                        _rot_tt(nc, T3[:, ss], ua[:, ss], wa[:, ss], 16)
                        oq = nc.gpsimd if (t + hw + sg) % 2 else nc.sync
                        oq.dma_start(out_d[ts, ss], T3[:, ss])
    nc.finalize()
    return nc


# ----------------------------------------------------------------- driver

_CACHE = {}


def kernel(X, params):
    X = np.ascontiguousarray(np.asarray(X, dtype=np.float32))
    params = np.asarray(params, dtype=np.float32)

    Uy, U8 = build_u_matrices(params)
    u_bf = np.ascontiguousarray(np.stack([
        Uy[:128], Uy[128:], U8[:128], U8[128:],
    ]).astype(ml_dtypes.bfloat16))
    wt, coef = build_weights(X)

    if "nc" not in _CACHE:
        _CACHE["nc"] = build_kernel()
    nc = _CACHE["nc"]

    ncols = BATCH // 128
    coef3 = coef.reshape(128, ncols, 6)
    in_maps = []
    for c in range(NCORES):
        bs = slice(c * BSH, (c + 1) * BSH)
        in_maps.append({
            "wt": np.ascontiguousarray(wt[:, :, :, bs]),
            "cf": np.ascontiguousarray(
                coef3[:, c * NTILES:(c + 1) * NTILES].reshape(
                    128, NTILES * 6)),
            "u": u_bf,
        })
    res = run_bass_kernel_spmd(nc, in_maps, list(range(NCORES)))
    out = np.concatenate([res.results[c]["out"] for c in range(NCORES)],
                         axis=0)
    # device columns are y-ordered; out[x] = dev[y(x)]
    out = out.astype(np.float32).reshape(BATCH, DIM, 2)
    return np.ascontiguousarray(out[:, _y_of_x(), :])
